# revision 1
# baseline (speedup 1.0000x reference)
"""Trainium2 Bass kernel for nn_Network_21998822490747 (embedding -> tiny LSTM -> vocab projection).

Strategy (8 NeuronCores, full inputs in / full output out):
  * Time-shard the T=4096 sequence: core c owns rows [c*512, (c+1)*512).
  * The LSTM recurrence is contractive (forget gate sigma(|x|<~1) <= 0.7), so each
    core runs S=32 parallel "streams" (time-chunks of L=16 steps) that each start
    W=48 steps early from zero state; after the warmup the state matches the exact
    scan to below fp32 noise (validated: max |h| err ~5e-8 for W>=24; W=32 used).
    Streams are vectorized along the SBUF free dimension, so one scan step is 7
    engine instructions covering all 32 streams.
  * All gate activations use a single tanh per step:
      sigmoid(x) = 0.5*(tanh(x/2)+1), handled with a per-partition scale vector
      and (t+1)-style fused scalar_tensor_tensor ops; state is kept as
      C=2c, h2=2h, with the 0.5 factors folded into w_hh and W_out host-side.
  * The memory-bound phase (this problem's target regime) is the [512,10+1] @
    [11, 50257] logits matmul per core: float32r matmuls (full PE rate) into
    PSUM, drained to SBUF alternating DVE/ACT, DMA'd to HBM at ~360GB/s/core.
  * The embedding gather runs on-device via one indirect DMA (2048 rows/core,
    incl. warmup rows) from the full table in device DRAM; an appended row V
    (least-squares solution of w_ih @ v = -(b_ih+b_hh)) makes out-of-range
    warmup steps exact no-ops so stream 0 starts from the true zero state.
"""

import os
import sys
import time

for _p in ("/opt/trn_rl_repo", "/root/.axon_site/_ro/trn_rl_repo"):
    if os.path.isdir(_p) and _p not in sys.path:
        sys.path.insert(0, _p)

import numpy as np

import concourse.bass as bass
import concourse.bacc as bacc
import concourse.mybir as mybir
import concourse.tile as tile
from concourse.bass import ts
from concourse.masks import make_identity

# Problem shapes
T, V, E, H, O = 4096, 128000, 256, 10, 50257
NCORES = 8
ROWS = T // NCORES        # 512 output rows per core

# Scan decomposition
S = 64                    # parallel streams per core
L = ROWS // S             # 16 real steps per stream
W = 16                    # warmup steps per stream
STEPS = L + W             # 64
NR = S * STEPS            # 2048 gathered rows per core
CB = NR // 128            # 16 gather column-blocks

# Logits tiling
OPAD = 51200              # O padded to 2 halves x 50 x 512
NQ = 2                    # wout partition groups (PE operand base must be 0/32/64)
QD = OPAD // NQ           # 25600
OC = QD // 512            # 50 moving chunks per half
STG = 5120                # staging tile columns per DMA batch
CPS = STG // 512          # psum chunks per staging tile

f32 = mybir.dt.float32
f32r = mybir.dt.float32r
i32 = mybir.dt.int32
AF = mybir.ActivationFunctionType
AL = mybir.AluOpType

GATE_PERM = np.r_[0:10, 10:20, 30:40, 20:30]   # [i, f, o, g] row order


def _tile_kernel(tc, nc, emb, idx, wihT, whhT, b40, wout, out, embg=None,
                 variant="gather16"):
    with (
        tc.tile_pool(name="const", bufs=1) as cpool,
        tc.tile_pool(name="work", bufs=1) as wpool,
    ):
        wih_sb = cpool.tile([128, 80], f32, tag="wih")
        whh_sb = cpool.tile([H, 40], f32, tag="whh")
        b40_sb = cpool.tile([40, 1], f32, tag="b40")
        ident = cpool.tile([128, 128], f32, tag="ident")
        wout_sb = cpool.tile([128, QD], f32r, tag="wout")

        nc.sync.dma_start(wih_sb[:, 0:40], wihT[0:128, :])
        nc.sync.dma_start(wih_sb[:, 40:80], wihT[128:256, :])
        nc.sync.dma_start(whh_sb[:], whhT[:])
        nc.sync.dma_start(b40_sb[:], b40[:])
        nc.sync.dma_start(wout_sb[:], wout[:])
        make_identity(nc, ident[:])

        # ---- gather + transpose + xg (gather tiles freed before logits)
        # xg32: streams-on-partitions layout, step t at cols [t*40, (t+1)*40)
        xg32 = wpool.tile([S, STEPS * 40], f32, tag="xg32")
        with tc.tile_pool(name="gath", bufs=1) as gpool:
            # gather 2048 embedding rows (incl. warmup rows)
            emb_raw = gpool.tile([128, CB * E], f32, tag="raw")
            if variant == "hostgather":
                nc.sync.dma_start(emb_raw[:], embg[:])
            elif variant == "gather1":
                idx_sb = cpool.tile([128, CB], i32, tag="idx")
                nc.sync.dma_start(idx_sb[:], idx[:])
                nc.gpsimd.indirect_dma_start(
                    out=emb_raw[:].rearrange("p (c e) -> p c e", e=E),
                    out_offset=None,
                    in_=emb[:, :],
                    in_offset=bass.IndirectOffsetOnAxis(ap=idx_sb[:, :], axis=0),
                )
            else:  # gather16: one [128,1]-offset indirect DMA per column block
                idx_sb = cpool.tile([128, CB], i32, tag="idx")
                nc.sync.dma_start(idx_sb[:], idx[:])
                for c in range(CB):
                    nc.gpsimd.indirect_dma_start(
                        out=emb_raw[:, c * E:(c + 1) * E],
                        out_offset=None,
                        in_=emb[:, :],
                        in_offset=bass.IndirectOffsetOnAxis(
                            ap=idx_sb[:, c:c + 1], axis=0),
                    )

            # transpose to emb^T layout [E, NR] (two 128-row halves)
            embT0 = gpool.tile([128, NR], f32, tag="embT0")
            embT1 = gpool.tile([128, NR], f32, tag="embT1")
            embTv = [embT0, embT1]
            with tc.tile_pool(name="pst", bufs=4, space="PSUM") as pst:
                for c in range(CB):
                    for e2 in range(2):
                        ps = pst.tile([128, 128], f32, tag="tp")
                        base = c * E + e2 * 128
                        nc.tensor.transpose(ps[:], emb_raw[:, base:base + 128],
                                            ident[:])
                        dst = embTv[e2][:].rearrange("q (p c) -> q p c", c=CB)[:, :, c]
                        nc.vector.tensor_copy(dst, ps[:])

            # xg40 = emb @ w_ih^T + bias (pre-scaled on host) -> [40, NR]
            xg40 = gpool.tile([40, NR], f32, tag="xg40")
            with tc.tile_pool(name="psx", bufs=2, space="PSUM") as psx:
                for n in range(NR // 512):
                    ps = psx.tile([40, 512], f32, tag="xg")
                    nc.tensor.matmul(ps[:], lhsT=wih_sb[:, 0:40],
                                     rhs=embT0[:, ts(n, 512)],
                                     start=True, stop=False)
                    nc.tensor.matmul(ps[:], lhsT=wih_sb[:, 40:80],
                                     rhs=embT1[:, ts(n, 512)],
                                     start=False, stop=True)
                    nc.scalar.activation(xg40[:, ts(n, 512)], ps[:], AF.Identity,
                                         bias=b40_sb[:, 0:1], scale=1.0)

            # transpose xg40 step-blocks [40, 32] -> xg32 blocks [32, 40]
            with tc.tile_pool(name="psx2", bufs=4, space="PSUM") as psx2:
                for t in range(STEPS):
                    ps = psx2.tile([S, 40], f32, tag="xt")
                    nc.tensor.transpose(ps[:], xg40[:, ts(t, S)], ident[0:40, 0:40])
                    nc.vector.tensor_copy(xg32[:, ts(t, 40)], ps[:])

        # ---- vectorized scan: 64 steps x 32 streams (streams on partitions)
        # th free-col layout: 0:40 tanh(gates i,f,o,g) | 40:50 C(=2c) | 50:60 tanh(c)
        hs = wpool.tile([11, (STEPS + 1) * S], f32, tag="hs")   # hT history + ones row
        th = wpool.tile([S, 60], f32, tag="th")
        gt = wpool.tile([S, 40], f32, tag="gt")
        uv = wpool.tile([S, 20], f32, tag="uv")
        h2 = wpool.tile([S, H], f32, tag="h2")
        nc.vector.memset(hs[:, :], 1.0)          # row 10 stays 1.0 (bias row)
        nc.vector.memset(th[:, 40:50], 0.0)      # C = 2c state
        nc.vector.memset(h2[:, :], 0.0)
        with (
            tc.tile_pool(name="psm", bufs=2, space="PSUM") as psm,
            tc.tile_pool(name="pst2", bufs=2, space="PSUM") as pst2,
        ):
            for t in range(STEPS + 1):
                # hT(t) = h2(t-1)^T  -> also the hs history used by logits
                pst_ = pst2.tile([H, S], f32, tag="ht")
                nc.tensor.transpose(pst_[:], h2[:, :], ident[0:S, 0:S])
                nc.vector.tensor_copy(hs[0:10, ts(t, S)], pst_[:])
                if t == STEPS:
                    break
                ps = psm.tile([S, 40], f32, tag="mv")
                nc.tensor.matmul(ps[:], lhsT=hs[0:10, ts(t, S)], rhs=whh_sb[:],
                                 start=True, stop=True)
                nc.vector.scalar_tensor_tensor(gt[:, :], ps[:], 1.0,
                                               xg32[:, ts(t, 40)], AL.mult, AL.add)
                nc.scalar.activation(th[:, 0:40], gt[:, :], AF.Tanh)
                # u = (th_i+1)*th_g ; v = (th_f+1)*C   (one fused op)
                nc.vector.scalar_tensor_tensor(uv[:, :], th[:, 0:20], 1.0,
                                               th[:, 30:50], AL.add, AL.mult)
                nc.vector.scalar_tensor_tensor(th[:, 40:50], uv[:, 10:20], 0.5,
                                               uv[:, 0:10], AL.mult, AL.add)
                nc.scalar.activation(th[:, 50:60], th[:, 40:50], AF.Tanh, scale=0.5)
                nc.vector.scalar_tensor_tensor(h2[:, :], th[:, 20:30], 1.0,
                                               th[:, 50:60], AL.add, AL.mult)

        # ---- logits: [11, 128]^T @ [11, 512] f32r matmuls, drain, DMA out
        hs_r = hs[:].rearrange("p (t s) -> p s t", s=S)    # [11, S, STEPS+1]
        with (
            tc.tile_pool(name="psl", bufs=8, space="PSUM") as psl,
            tc.tile_pool(name="stage", bufs=3) as stpool,
            tc.tile_pool(name="statp", bufs=2) as statpool,
        ):
            SPB = 128 // L           # streams per 128-row block
            for blk in range(ROWS // 128):
                s0 = blk * SPB
                # PE needs stationary+moving at the same base partition; wout
                # lives in NQ partition groups, so replicate the tiny hs block
                # into each group of statq.
                statq = statpool.tile([128, 128], f32r, tag="statq")
                for q in range(NQ):
                    nc.vector.tensor_copy(
                        statq[64 * q:64 * q + 11, :].rearrange(
                            "p (a b) -> p a b", b=L),
                        hs_r[0:11, s0:s0 + SPB, W + 1:W + 1 + L])
                for q in range(NQ):
                    stat = statq[64 * q:64 * q + 11, :]
                    stage = None
                    for oc in range(OC):
                        ps = psl.tile([128, 512], f32, tag="lg")
                        nc.tensor.matmul(
                            ps[:], lhsT=stat,
                            rhs=wout_sb[64 * q:64 * q + 11, ts(oc, 512)],
                            start=True, stop=True)
                        if oc % CPS == 0:
                            stage = stpool.tile([128, STG], f32, tag="stg")
                        if oc & 1:
                            nc.scalar.copy(stage[:, ts(oc % CPS, 512)], ps[:])
                        else:
                            nc.vector.tensor_copy(stage[:, ts(oc % CPS, 512)], ps[:])
                        if oc % CPS == CPS - 1:
                            col = q * QD + (oc // CPS) * STG
                            nc.sync.dma_start(
                                out[ts(blk, 128), col:col + STG], stage[:])


def build_program_real(variant="gather16"):
    nc = bacc.Bacc("TRN2", target_bir_lowering=False, debug=False,
                   enable_asserts=False)
    emb_ap = idx_ap = embg_ap = None
    if variant == "hostgather":
        embg_ap = nc.dram_tensor("embg", [128, CB * E], f32,
                                 kind="ExternalInput").ap()
    else:
        emb_ap = nc.dram_tensor("emb", [V + 1, E], f32, kind="ExternalInput").ap()
        idx_ap = nc.dram_tensor("idx", [128, CB], i32, kind="ExternalInput").ap()
    wih_d = nc.dram_tensor("wihT", [E, 40], f32, kind="ExternalInput")
    whh_d = nc.dram_tensor("whhT05", [H, 40], f32, kind="ExternalInput")
    b40_d = nc.dram_tensor("bias40", [40, 1], f32, kind="ExternalInput")
    wout_d = nc.dram_tensor("wout", [128, QD], f32r, kind="ExternalInput")
    out_d = nc.dram_tensor("out", [ROWS, OPAD], f32, kind="ExternalOutput")

    with tile.TileContext(nc) as tc:
        _tile_kernel(tc, nc, emb_ap, idx_ap, wih_d.ap(), whh_d.ap(),
                     b40_d.ap(), wout_d.ap(), out_d.ap(), embg=embg_ap,
                     variant=variant)
    nc.compile()
    return nc


def prep_host(inputs):
    """Shared (core-independent) prepped arrays + per-core index tables."""
    x = np.asarray(inputs["x"]).astype(np.int64)
    embedding = np.asarray(inputs["embedding"], dtype=np.float32)
    w_ih = np.asarray(inputs["w_ih"], dtype=np.float32)
    w_hh = np.asarray(inputs["w_hh"], dtype=np.float32)
    b_ih = np.asarray(inputs["b_ih"], dtype=np.float32)
    b_hh = np.asarray(inputs["b_hh"], dtype=np.float32)
    W_out = np.asarray(inputs["W_out"], dtype=np.float32)
    b_out = np.asarray(inputs["b_out"], dtype=np.float32)

    p = GATE_PERM
    # gate scale: sigmoid(x) = 0.5*(tanh(x/2)+1) -> scale i,f,o preacts by 0.5,
    # folded into w_ih / bias; w_hh additionally gets the h2=2h factor (x0.5).
    gsc = np.concatenate([np.full(30, 0.5), np.ones(10)]).astype(np.float32)
    w_ih_p = w_ih[p] * gsc[:, None]
    bias40 = ((b_ih + b_hh)[p] * gsc).astype(np.float32)
    whh05 = (w_hh[p].T * (0.5 * gsc)[None, :]).astype(np.float32)   # [10, 40]
    wihT = np.ascontiguousarray(w_ih_p.T).astype(np.float32)        # [256, 40]

    # Padding row V: w_ih @ v = -(b_ih + b_hh)  => xg row == 0 for padded steps
    v, *_ = np.linalg.lstsq(w_ih.astype(np.float64), -(b_ih + b_hh).astype(np.float64),
                            rcond=None)
    emb_aug = np.concatenate([embedding, v[None, :].astype(np.float32)], axis=0)

    woutp = np.zeros((128, QD), np.float32)
    Wt = np.zeros((OPAD, H), np.float32)
    Wt[:O] = 0.5 * W_out
    bo = np.zeros(OPAD, np.float32)
    bo[:O] = b_out
    for q in range(NQ):
        woutp[64 * q:64 * q + 10, :] = Wt[q * QD:(q + 1) * QD].T
        woutp[64 * q + 10, :] = bo[q * QD:(q + 1) * QD]

    idx_cores = []
    embg_cores = []
    for c in range(NCORES):
        j = np.arange(NR)
        t = j // S
        s = j % S
        g_r = c * ROWS + s * L - W + t
        val = np.where(g_r < 0, V, x[np.clip(g_r, 0, T - 1)])
        # tile position (p, cb) holds gather row j = p*CB + cb
        idx_cores.append(val.reshape(128, CB).astype(np.int32))
        embg_cores.append(emb_aug[val].reshape(128, CB * E).astype(np.float32))

    shared = {
        "emb": emb_aug,
        "wihT": wihT,
        "whhT05": whh05,
        "bias40": bias40.reshape(40, 1),
        "wout": woutp,
    }
    return shared, idx_cores, embg_cores


def in_maps_for(inputs):
    shared, idx_cores, embg_cores = prep_host(inputs)
    return [{**shared, "idx": idx_cores[c], "embg": embg_cores[c]}
            for c in range(NCORES)]


_EXEC_CACHE = {}


def _get_exec(variant="gather16"):
    """Build (once) the compiled 8-core PJRT executable and metadata."""
    if variant in _EXEC_CACHE:
        return _EXEC_CACHE[variant]

    import jax
    from jax.sharding import Mesh, PartitionSpec, NamedSharding
    try:
        from jax.experimental.shard_map import shard_map
    except ImportError:
        from jax import shard_map
    from concourse import bass2jax

    bass2jax.install_neuronx_cc_hook()
    nc = build_program_real(variant)

    pname = nc.partition_id_tensor.name if nc.partition_id_tensor else None
    in_names, out_names, out_avals = [], [], []
    for alloc in nc.m.functions[0].allocations:
        if not isinstance(alloc, mybir.MemoryLocationSet):
            continue
        name = alloc.memorylocations[0].name
        if alloc.kind == "ExternalInput":
            if name != pname:
                in_names.append(name)
        elif alloc.kind == "ExternalOutput":
            out_names.append(name)
            out_avals.append(jax.core.ShapedArray(
                tuple(alloc.tensor_shape), mybir.dt.np(alloc.dtype)))
    n_params = len(in_names)
    all_names = in_names + out_names + ([pname] if pname else [])

    def _body(*args):
        operands = list(args)
        if pname is not None:
            operands.append(bass2jax.partition_id_tensor())
        outs = bass2jax._bass_exec_p.bind(
            *operands,
            out_avals=tuple(out_avals),
            in_names=tuple(all_names),
            out_names=tuple(out_names),
            lowering_input_output_aliases=(),
            sim_require_finite=False,
            sim_require_nnan=False,
            nc=nc,
        )
        return tuple(outs)

    devices = jax.devices()[:NCORES]
    mesh = Mesh(np.asarray(devices), ("core",))
    spec_in = (PartitionSpec("core"),) * (n_params + len(out_names))
    spec_out = (PartitionSpec("core"),) * len(out_names)
    donate = tuple(range(n_params, n_params + len(out_names)))
    fn = jax.jit(
        shard_map(_body, mesh=mesh, in_specs=spec_in, out_specs=spec_out,
                  check_rep=False),
        donate_argnums=donate, keep_unused=True)

    res = {
        "jax": jax, "mesh": mesh, "NamedSharding": NamedSharding,
        "PartitionSpec": PartitionSpec, "fn": fn, "nc": nc,
        "in_names": in_names, "out_names": out_names, "out_avals": out_avals,
        "devices": devices,
    }
    _EXEC_CACHE[variant] = res
    return res


def _place_inputs(ex, in_maps):
    """Transfer per-core input shards to the 8 devices, return global arrays."""
    jax = ex["jax"]
    NamedSharding, PartitionSpec = ex["NamedSharding"], ex["PartitionSpec"]
    sharding = NamedSharding(ex["mesh"], PartitionSpec("core"))
    placed = []
    for name in ex["in_names"]:
        shards = [np.asarray(in_maps[c][name]) for c in range(NCORES)]
        per_dev = [jax.device_put(s, d) for s, d in zip(shards, ex["devices"])]
        gshape = (NCORES * shards[0].shape[0],) + shards[0].shape[1:]
        placed.append(jax.make_array_from_single_device_arrays(
            gshape, sharding, per_dev))
    jax.block_until_ready(placed)
    return placed, sharding


def _zero_outs(ex, sharding):
    import jax.numpy as jnp
    outs = []
    for av in ex["out_avals"]:
        gshape = (NCORES * av.shape[0],) + av.shape[1:]
        outs.append(jnp.zeros(gshape, av.dtype, device=sharding))
    ex["jax"].block_until_ready(outs)
    return outs


def run_hw(inputs, time_iters=0, variant=None):
    """Run on the 8 NeuronCores. Returns (full_output, wall_times_s)."""
    if variant is None:
        variant = os.environ.get("KERNEL_VARIANT", "gather16")
    ex = _get_exec(variant)
    jax = ex["jax"]
    in_maps = in_maps_for(inputs)
    placed, sharding = _place_inputs(ex, in_maps)

    zouts = _zero_outs(ex, sharding)
    res = ex["fn"](*placed, *zouts)
    jax.block_until_ready(res)
    out_global = np.asarray(res[0])          # [8*512, OPAD]

    times = []
    for _ in range(time_iters):
        zouts = _zero_outs(ex, sharding)
        t0 = time.perf_counter()
        r = ex["fn"](*placed, *zouts)
        jax.block_until_ready(r)
        times.append(time.perf_counter() - t0)

    full = out_global[:, :O].reshape(T, 1, O).astype(np.float32)
    return full, times


def kernel(**inputs):
    out, _ = run_hw(inputs, time_iters=0)
    return out


# ---------------------------------------------------------------- dev helpers

def sim_check(inputs, core=0, variant="gather16"):
    """Run core `core`'s program in CoreSim, return its [512, OPAD] output."""
    from concourse.bass_interp import CoreSim
    nc = build_program_real(variant)
    sim = CoreSim(nc, trace=False, require_finite=False, require_nnan=False)
    in_maps = in_maps_for(inputs)
    for name, arr in in_maps[core].items():
        try:
            sim.tensor(name)[:] = arr
        except KeyError:
            pass
    sim.simulate(check_with_hw=False)
    return np.array(sim.tensor("out"))


def timeline(variant="gather16"):
    from concourse.timeline_sim import TimelineSim
    nc = build_program_real(variant)
    tl = TimelineSim(nc, trace=False)
    tl.simulate()
    return tl


def probe_floor(iters=5):
    """Wall-time floor of the 8-core dispatch path using a trivial NEFF."""
    import jax
    from jax.sharding import Mesh, PartitionSpec, NamedSharding
    try:
        from jax.experimental.shard_map import shard_map
    except ImportError:
        from jax import shard_map
    from concourse import bass2jax
    bass2jax.install_neuronx_cc_hook()

    nc = bacc.Bacc("TRN2", target_bir_lowering=False, debug=False,
                   enable_asserts=False)
    pin = nc.dram_tensor("pin", [128, 128], f32, kind="ExternalInput")
    pout = nc.dram_tensor("pout", [128, 128], f32, kind="ExternalOutput")
    with tile.TileContext(nc) as tc:
        with tc.tile_pool(name="p", bufs=1) as pool:
            t = pool.tile([128, 128], f32, tag="t")
            nc.sync.dma_start(t[:], pin.ap()[:])
            nc.sync.dma_start(pout.ap()[:], t[:])
    nc.compile()

    pname = nc.partition_id_tensor.name if nc.partition_id_tensor else None
    all_names = ["pin", "pout"] + ([pname] if pname else [])

    def _body(a, z):
        ops = [a, z]
        if pname is not None:
            ops.append(bass2jax.partition_id_tensor())
        return tuple(bass2jax._bass_exec_p.bind(
            *ops, out_avals=(jax.core.ShapedArray((128, 128), np.float32),),
            in_names=tuple(all_names), out_names=("pout",),
            lowering_input_output_aliases=(),
            sim_require_finite=False, sim_require_nnan=False, nc=nc))

    devices = jax.devices()[:NCORES]
    mesh = Mesh(np.asarray(devices), ("core",))
    sharding = NamedSharding(mesh, PartitionSpec("core"))
    fn = jax.jit(shard_map(_body, mesh=mesh,
                           in_specs=(PartitionSpec("core"),) * 2,
                           out_specs=(PartitionSpec("core"),),
                           check_rep=False), keep_unused=True)
    import jax.numpy as jnp
    a = jax.device_put(np.zeros((NCORES * 128, 128), np.float32), sharding)
    z = jnp.zeros((NCORES * 128, 128), np.float32, device=sharding)
    jax.block_until_ready([a, z])
    r = fn(a, z); jax.block_until_ready(r)   # warm

    def timed(reps):
        best = float("inf")
        for _ in range(iters):
            t0 = time.perf_counter()
            r = None
            for _ in range(reps):
                r = fn(a, z)
            jax.block_until_ready(r)
            best = min(best, time.perf_counter() - t0)
        return best

    w1 = timed(1)
    wk = timed(50)
    return (wk - w1) / 49.0, wk, w1

def run_hw_async(inputs, k=50, iters=3, variant="gather16"):
    """Per-exec time via async pipelining: submit k executions without
    intermediate blocking; marginal cost per call ~= device exec time if the
    runtime queues them. Returns (per_exec_s, wall_k, wall_1)."""
    import jax
    from jax.sharding import PartitionSpec
    try:
        from jax.experimental.shard_map import shard_map
    except ImportError:
        from jax import shard_map
    from concourse import bass2jax
    ex = _get_exec(variant)
    nc = ex["nc"]
    pname = nc.partition_id_tensor.name if nc.partition_id_tensor else None
    in_names, out_names, out_avals = ex["in_names"], ex["out_names"], ex["out_avals"]
    all_names = in_names + out_names + ([pname] if pname else [])

    def _body(*args):
        ops = list(args)
        if pname is not None:
            ops.append(bass2jax.partition_id_tensor())
        return tuple(bass2jax._bass_exec_p.bind(
            *ops, out_avals=tuple(out_avals), in_names=tuple(all_names),
            out_names=tuple(out_names), lowering_input_output_aliases=(),
            sim_require_finite=False, sim_require_nnan=False, nc=nc))

    nin = len(in_names) + len(out_names)
    fn = jax.jit(shard_map(_body, mesh=ex["mesh"],
                           in_specs=(PartitionSpec("core"),) * nin,
                           out_specs=(PartitionSpec("core"),) * len(out_names),
                           check_rep=False), keep_unused=True)  # no donation

    in_maps = in_maps_for(inputs)
    placed, sharding = _place_inputs(ex, in_maps)
    zouts = _zero_outs(ex, sharding)
    r = fn(*placed, *zouts); jax.block_until_ready(r)   # warm

    def timed(reps):
        best = float("inf")
        for _ in range(iters):
            t0 = time.perf_counter()
            r = None
            for _ in range(reps):
                r = fn(*placed, *zouts)
            jax.block_until_ready(r)
            best = min(best, time.perf_counter() - t0)
        return best

    w1 = timed(1)
    wk = timed(k)
    return (wk - w1) / (k - 1), wk, w1



# revision 15
# speedup vs baseline: 1.5390x; 1.5390x over previous
"""Trainium2 Bass kernel for nn_Network_21998822490747 (embedding -> tiny LSTM -> vocab projection).

Strategy (8 NeuronCores, full inputs in / full output out):
  * Time-shard the T=4096 sequence: core c owns rows [c*512, (c+1)*512).
  * The contractive LSTM recurrence runs as S=128 parallel streams (time-chunks
    of L=4 steps) that each start W=16 steps early from zero state; after the
    warmup the state matches the exact scan to fp32 noise. Streams are
    vectorized along SBUF partitions, so one scan step is ~7 engine
    instructions covering all 128 streams.
  * All gate activations use a single tanh per step (sigmoid(x)=0.5(tanh(x/2)+1)
    with scales folded into weights host-side; state kept as C=2c, h2=2h).
  * Memory-bound phase = the [512,10] x [10, 50257] logits matmul per core:
      - W_out/bias split into fp8e4 hi+lo components (host), hs split into
        fp8 hi+lo on device -> fp8 DoubleRow matmuls (2 cols/cycle, K=16x2)
        reproduce bf16-accuracy logits at twice the PE rate.
      - PSUM f32 results are drained to bf16 staging tiles by all three data
        engines (DVE/ACT/Pool, weighted by their throughput), then DMA'd to a
        bf16 output tensor; the host upcasts to f32 (rel-err ~4e-3 << 2e-2).
  * Embedding table stored bf16 on device (halves gather traffic); an appended
    row V (least-squares solution of w_ih @ v = -(b_ih+b_hh)) makes pre-start
    warmup steps exact no-ops so stream 0 starts from the true zero state.
"""

import os
import sys
import time

for _p in ("/opt/trn_rl_repo", "/root/.axon_site/_ro/trn_rl_repo"):
    if os.path.isdir(_p) and _p not in sys.path:
        sys.path.insert(0, _p)

import numpy as np
import ml_dtypes

import concourse.bass as bass
import concourse.bacc as bacc
import concourse.mybir as mybir
import concourse.tile as tile
from concourse.bass import ts
from concourse.masks import make_identity

# Problem shapes
T, V, E, H, O = 4096, 128000, 256, 10, 50257
NCORES = 8
ROWS = T // NCORES        # 512 output rows per core

# Scan decomposition
S = 128                   # parallel streams per core (on SBUF partitions)
L = ROWS // S             # 4 real steps per stream
W = 16                    # warmup steps per stream
STEPS = L + W             # 20
NR = S * STEPS            # 2560 gathered rows per core
CB = NR // 128            # 20 gather column-blocks

# Logits tiling
OPAD = 51200              # O padded to 100 x 512
OC = OPAD // 512          # 512-col chunks per 128-row block
PSB = 2048                # psum drain tile cols (4 chunks / 4 banks)
NPT = OPAD // PSB         # 25 psum tiles per block
STG = 10240               # staging tile cols (5 psum tiles)
NST = OPAD // STG         # 5 staging batches per block

f32 = mybir.dt.float32
bf16 = mybir.dt.bfloat16
fp8 = mybir.dt.float8e4
i32 = mybir.dt.int32
AF = mybir.ActivationFunctionType
AL = mybir.AluOpType
PM = mybir.MatmulPerfMode

NP_BF16 = ml_dtypes.bfloat16
NP_FP8 = ml_dtypes.float8_e4m3

GATE_PERM = np.r_[0:10, 10:20, 30:40, 20:30]   # [i, f, o, g] row order

# drain-engine split: target shares solving for equal per-engine busy time
# given per-tile costs (DVE 2258ns, ACT ~1890ns) and each engine's fixed
# (non-drain) work. GPSIMD/Pool cannot read PSUM on TRN2.
_DRAIN_W = {"v": 0.45, "a": 0.55}


def _drain_pattern(n):
    tot = sum(_DRAIN_W.values())
    acc = {k: 0.0 for k in _DRAIN_W}
    out = []
    for _ in range(n):
        for k in _DRAIN_W:
            acc[k] += _DRAIN_W[k] / tot
        pick = max(acc, key=lambda k: acc[k])
        acc[pick] -= 1.0
        out.append(pick)
    return out


def _tile_kernel(tc, nc, emb, idx, wihT, whhT, b40, wout, out):
    with (
        tc.tile_pool(name="const", bufs=1) as cpool,
        tc.tile_pool(name="work", bufs=1) as wpool,
    ):
        wih_sb = cpool.tile([128, 80], bf16, tag="wih")
        whh_sb = cpool.tile([H, 40], bf16, tag="whh")
        b40_sb = cpool.tile([40, 1], f32, tag="b40")
        ident = cpool.tile([128, 128], f32, tag="ident")
        identb = cpool.tile([128, 128], bf16, tag="identb")
        wout_sb = cpool.tile([16, 2 * OPAD], fp8, tag="wout")
        idx_sb = cpool.tile([128, CB], i32, tag="idx")

        nc.sync.dma_start(wih_sb[:, 0:40], wihT[0:128, :])
        nc.sync.dma_start(wih_sb[:, 40:80], wihT[128:256, :])
        nc.sync.dma_start(whh_sb[:], whhT[:])
        nc.sync.dma_start(b40_sb[:], b40[:])
        nc.sync.dma_start(wout_sb[:], wout[:])
        nc.sync.dma_start(idx_sb[:], idx[:])
        make_identity(nc, ident[:])
        make_identity(nc, identb[:])

        # hs history: h2 (=2h) per step, bf16, streams-major inside each step
        hs = wpool.tile([H, (STEPS + 1) * S], bf16, tag="hs")

        # ---- gather + transpose + xg (gather tiles freed before logits)
        # xg128: streams-on-partitions, step t at cols [t*40, (t+1)*40)
        xg128 = wpool.tile([S, STEPS * 40], f32, tag="xg128")
        with tc.tile_pool(name="gath", bufs=1) as gpool:
            emb_raw = gpool.tile([128, CB * E], bf16, tag="raw")
            for c in range(CB):
                nc.gpsimd.indirect_dma_start(
                    out=emb_raw[:, c * E:(c + 1) * E],
                    out_offset=None,
                    in_=emb[:, :],
                    in_offset=bass.IndirectOffsetOnAxis(
                        ap=idx_sb[:, c:c + 1], axis=0),
                )

            # transpose to emb^T layout [E, NR] (two 128-row halves), bf16
            embT0 = gpool.tile([128, NR], bf16, tag="embT0")
            embT1 = gpool.tile([128, NR], bf16, tag="embT1")
            embTv = [embT0, embT1]
            with tc.tile_pool(name="pst", bufs=2, space="PSUM") as pst:
                for e2 in range(2):
                    for c0 in range(0, CB, 4):
                        ps = pst.tile([128, 512], bf16, tag="tp")
                        for k in range(4):
                            c = c0 + k
                            base = c * E + e2 * 128
                            nc.tensor.transpose(
                                ps[:, ts(k, 128)],
                                emb_raw[:, base:base + 128], identb[:])
                        # psum (c-major, p) -> embT free layout (p*CB + c)
                        # all-bf16 copy: eligible for the DVE 2x mode
                        dst = embTv[e2][:].rearrange(
                            "q (p c) -> q c p", c=CB)[:, c0:c0 + 4, :]
                        src = ps[:].rearrange("q (c p) -> q c p", c=4)
                        nc.vector.tensor_copy(dst, src)

            # xg40 = emb @ w_ih^T + bias (pre-scaled on host) -> [40, NR] f32
            xg40 = gpool.tile([40, NR], f32, tag="xg40")
            with tc.tile_pool(name="psx", bufs=2, space="PSUM") as psx:
                for n in range(NR // 512):
                    ps = psx.tile([40, 512], f32, tag="xg")
                    nc.tensor.matmul(ps[:], lhsT=wih_sb[:, 0:40],
                                     rhs=embT0[:, ts(n, 512)],
                                     start=True, stop=False)
                    nc.tensor.matmul(ps[:], lhsT=wih_sb[:, 40:80],
                                     rhs=embT1[:, ts(n, 512)],
                                     start=False, stop=True)
                    nc.scalar.activation(xg40[:, ts(n, 512)], ps[:], AF.Identity,
                                         bias=b40_sb[:, 0:1], scale=1.0)

            # transpose xg40 step-blocks [40, S] -> xg128 blocks [S, 40]
            with tc.tile_pool(name="psx2", bufs=2, space="PSUM") as psx2:
                for t0 in range(0, STEPS, 4):
                    ps = psx2.tile([S, 160], f32, tag="xt")
                    for k in range(4):
                        nc.tensor.transpose(ps[:, ts(k, 40)],
                                            xg40[:, ts(t0 + k, S)],
                                            ident[0:40, 0:40])
                    eng = nc.vector.tensor_copy if (t0 // 4) % 2 else nc.scalar.copy
                    eng(xg128[:, t0 * 40:(t0 + 4) * 40], ps[:])

        # ---- vectorized scan: STEPS x S streams (streams on partitions)
        # th free-col layout: 0:40 tanh(gates i,f,o,g) | 40:50 C(=2c) | 50:60 tanh(c)
        th = wpool.tile([S, 60], f32, tag="th")
        gt = wpool.tile([S, 40], f32, tag="gt")
        uv = wpool.tile([S, 20], f32, tag="uv")
        h2 = wpool.tile([S, H], f32, tag="h2")
        nc.gpsimd.memset(th[:, 40:50], 0.0)      # C = 2c state
        nc.gpsimd.memset(h2[:, :], 0.0)
        with (
            tc.tile_pool(name="psm", bufs=2, space="PSUM") as psm,
            tc.tile_pool(name="pst2", bufs=2, space="PSUM") as pst2,
        ):
            for t in range(STEPS + 1):
                # hT(t) = h2(t-1)^T  -> the hs history used by logits (bf16)
                pst_ = pst2.tile([H, S], f32, tag="ht")
                nc.tensor.transpose(pst_[:], h2[:, :], ident[0:S, 0:S])
                nc.vector.tensor_copy(hs[:, ts(t, S)], pst_[:])
                if t == STEPS:
                    break
                ps = psm.tile([S, 40], f32, tag="mv")
                nc.tensor.matmul(ps[:], lhsT=hs[:, ts(t, S)], rhs=whh_sb[:],
                                 start=True, stop=True)
                nc.vector.scalar_tensor_tensor(gt[:, :], ps[:], 1.0,
                                               xg128[:, ts(t, 40)], AL.mult, AL.add)
                nc.scalar.activation(th[:, 0:40], gt[:, :], AF.Tanh)
                # u = (th_i+1)*th_g ; v = (th_f+1)*C   (one fused op)
                nc.vector.scalar_tensor_tensor(uv[:, :], th[:, 0:20], 1.0,
                                               th[:, 30:50], AL.add, AL.mult)
                nc.vector.scalar_tensor_tensor(th[:, 40:50], uv[:, 10:20], 0.5,
                                               uv[:, 0:10], AL.mult, AL.add)
                nc.scalar.activation(th[:, 50:60], th[:, 40:50], AF.Tanh, scale=0.5)
                nc.vector.scalar_tensor_tensor(h2[:, :], th[:, 20:30], 1.0,
                                               th[:, 50:60], AL.add, AL.mult)

        # ---- fp8 hi/lo split of hs output rows (h2 = h8 + d8)
        # output row r = s*L + u  <-  hs step W+1+u, stream s
        hs_r = hs[:].rearrange("p (t s) -> p s t", s=S)[:, :, W + 1:W + 1 + L]
        h8 = wpool.tile([H, ROWS], fp8, tag="h8")
        h8b = wpool.tile([H, ROWS], bf16, tag="h8b")
        d8 = wpool.tile([H, ROWS], fp8, tag="d8")
        statq = wpool.tile([16, 4 * 256], fp8, tag="statq")
        nc.vector.tensor_copy(h8[:], hs_r)
        nc.vector.tensor_copy(h8b[:], h8[:])
        nc.vector.tensor_tensor(d8[:], hs_r, h8b[:], AL.subtract)
        # statq layout: [16, (blk 4, i 2, m 128)]; memset 1.0 covers bias rows
        # (SBUF-only assembly runs on Pool to keep DVE/ACT free for drains)
        nc.gpsimd.memset(statq[:, :], 1.0)
        st4 = statq[:].rearrange("p (b i m) -> p b i m", i=2, m=128)
        h8_4 = h8[:].rearrange("p (b m) -> p b m", m=128)
        d8_4 = d8[:].rearrange("p (b m) -> p b m", m=128)
        nc.gpsimd.tensor_copy(st4[0:10, :, 0, :], h8_4)
        nc.gpsimd.tensor_copy(st4[0:10, :, 1, :], h8_4)
        # engines can't start at partition 11; DMA has no such restriction
        nc.sync.dma_start(st4[11:16, :, 0, :], d8_4[0:5])
        nc.sync.dma_start(st4[11:16, :, 1, :], d8_4[5:10])

        # ---- logits: fp8 DoubleRow matmuls, DVE/ACT drain, bf16 DMA out
        # wout packed per 512-chunk: [16, (oc, i, 512)] so AP strides stay
        # within the 16-bit ISA field (a global i-stride of OPAD=51200 is not
        # encodable)
        drain_eng = {"v": nc.vector.tensor_copy,
                     "a": nc.scalar.copy}
        pattern = _drain_pattern(4 * NPT)
        pi = 0
        with (
            tc.tile_pool(name="psl", bufs=2, space="PSUM") as psl,
            tc.tile_pool(name="stage", bufs=3) as stpool,
        ):
            for blk in range(ROWS // 128):
                lhsT = statq[:, blk * 256:(blk + 1) * 256].rearrange(
                    "p (i m) -> p i m", i=2)
                stage = None
                for pt in range(NPT):
                    ps = psl.tile([128, PSB], f32, tag="lg")
                    for k in range(4):
                        oc = pt * 4 + k
                        rhs = wout_sb[:, oc * 1024:(oc + 1) * 1024].rearrange(
                            "p (i n) -> p i n", i=2)
                        nc.tensor.matmul(
                            ps[:, ts(k, 512)], lhsT=lhsT,
                            rhs=rhs,
                            start=True, stop=True, perf_mode=PM.DoubleRow)
                    if pt % 5 == 0:
                        stage = stpool.tile([128, STG], bf16, tag="stg")
                    drain_eng[pattern[pi]](stage[:, ts(pt % 5, PSB)], ps[:])
                    pi += 1
                    if pt % 5 == 4:
                        col = (pt // 5) * STG
                        nc.sync.dma_start(
                            out[ts(blk, 128), col:col + STG], stage[:])


def build_program_real(variant=None):
    nc = bacc.Bacc("TRN2", target_bir_lowering=False, debug=False,
                   enable_asserts=False)
    emb_ap = nc.dram_tensor("emb", [V + 1, E], bf16, kind="ExternalInput").ap()
    idx_ap = nc.dram_tensor("idx", [128, CB], i32, kind="ExternalInput").ap()
    wih_d = nc.dram_tensor("wihT", [E, 40], bf16, kind="ExternalInput")
    whh_d = nc.dram_tensor("whhT05", [H, 40], bf16, kind="ExternalInput")
    b40_d = nc.dram_tensor("bias40", [40, 1], f32, kind="ExternalInput")
    wout_d = nc.dram_tensor("wout", [16, 2 * OPAD], fp8, kind="ExternalInput")
    out_d = nc.dram_tensor("out", [ROWS, OPAD], bf16, kind="ExternalOutput")

    with tile.TileContext(nc) as tc:
        _tile_kernel(tc, nc, emb_ap, idx_ap, wih_d.ap(), whh_d.ap(),
                     b40_d.ap(), wout_d.ap(), out_d.ap())
    nc.compile()
    return nc


def prep_host(inputs):
    """Shared (core-independent) prepped arrays + per-core index tables."""
    x = np.asarray(inputs["x"]).astype(np.int64)
    embedding = np.asarray(inputs["embedding"], dtype=np.float32)
    w_ih = np.asarray(inputs["w_ih"], dtype=np.float32)
    w_hh = np.asarray(inputs["w_hh"], dtype=np.float32)
    b_ih = np.asarray(inputs["b_ih"], dtype=np.float32)
    b_hh = np.asarray(inputs["b_hh"], dtype=np.float32)
    W_out = np.asarray(inputs["W_out"], dtype=np.float32)
    b_out = np.asarray(inputs["b_out"], dtype=np.float32)

    p = GATE_PERM
    # gate scale: sigmoid(x) = 0.5*(tanh(x/2)+1) -> scale i,f,o preacts by 0.5,
    # folded into w_ih / bias; w_hh additionally gets the h2=2h factor (x0.5).
    gsc = np.concatenate([np.full(30, 0.5), np.ones(10)]).astype(np.float32)
    w_ih_p = w_ih[p] * gsc[:, None]
    bias40 = ((b_ih + b_hh)[p] * gsc).astype(np.float32)
    whh05 = (w_hh[p].T * (0.5 * gsc)[None, :]).astype(NP_BF16)     # [10, 40]
    wihT = np.ascontiguousarray(w_ih_p.T).astype(NP_BF16)          # [256, 40]

    # Padding row V: w_ih @ v = -(b_ih + b_hh)  => xg row == 0 for padded steps
    v, *_ = np.linalg.lstsq(w_ih.astype(np.float64),
                            -(b_ih + b_hh).astype(np.float64), rcond=None)
    emb_aug = np.concatenate(
        [embedding, v[None, :].astype(np.float32)], axis=0).astype(NP_BF16)

    # fp8 hi/lo split of 0.5*W_out (h2=2h folds the 2x) and b_out
    Wt = np.zeros((OPAD, H), np.float32)
    Wt[:O] = 0.5 * W_out
    bo = np.zeros(OPAD, np.float32)
    bo[:O] = b_out
    W_hi = Wt.astype(NP_FP8)
    W_lo = (Wt - W_hi.astype(np.float32)).astype(NP_FP8)
    b_hi = bo.astype(NP_FP8)
    b_lo = (bo - b_hi.astype(np.float32)).astype(NP_FP8)
    # woutp[p, i, n]: i=0 {rows0-9: W_hi, 10: b_hi, 11-15: W_hi[:, :5] (d8 pair)}
    #                i=1 {rows0-9: W_lo, 10: b_lo, 11-15: W_hi[:, 5:]}
    woutp = np.zeros((16, 2, OPAD), NP_FP8)
    woutp[0:10, 0, :] = W_hi.T
    woutp[10, 0, :] = b_hi
    woutp[11:16, 0, :] = W_hi.T[0:5]
    woutp[0:10, 1, :] = W_lo.T
    woutp[10, 1, :] = b_lo
    woutp[11:16, 1, :] = W_hi.T[5:10]
    # interleave hi/lo per 512-col chunk: [16, (oc, i, 512)]
    woutp = np.ascontiguousarray(
        woutp.reshape(16, 2, OPAD // 512, 512).transpose(0, 2, 1, 3)
    ).reshape(16, 2 * OPAD)

    idx_cores = []
    for c in range(NCORES):
        j = np.arange(NR)
        t = j // S
        s = j % S
        g_r = c * ROWS + s * L - W + t
        val = np.where(g_r < 0, V, x[np.clip(g_r, 0, T - 1)])
        # tile position (p, cb) holds gather row j = p*CB + cb
        idx_cores.append(val.reshape(128, CB).astype(np.int32))

    shared = {
        "emb": emb_aug,
        "wihT": wihT,
        "whhT05": whh05,
        "bias40": np.asarray(bias40).reshape(40, 1),
        "wout": woutp,
    }
    return shared, idx_cores


def in_maps_for(inputs):
    shared, idx_cores = prep_host(inputs)
    return [{**shared, "idx": idx_cores[c]} for c in range(NCORES)]


_EXEC_CACHE = {}


def _get_exec(variant=None):
    """Build (once) the compiled 8-core PJRT executable and metadata."""
    if variant in _EXEC_CACHE:
        return _EXEC_CACHE[variant]

    import jax
    from jax.sharding import Mesh, PartitionSpec, NamedSharding
    try:
        from jax.experimental.shard_map import shard_map
    except ImportError:
        from jax import shard_map
    from concourse import bass2jax

    bass2jax.install_neuronx_cc_hook()
    nc = build_program_real(variant)

    pname = nc.partition_id_tensor.name if nc.partition_id_tensor else None
    in_names, out_names, out_avals = [], [], []
    for alloc in nc.m.functions[0].allocations:
        if not isinstance(alloc, mybir.MemoryLocationSet):
            continue
        name = alloc.memorylocations[0].name
        if alloc.kind == "ExternalInput":
            if name != pname:
                in_names.append(name)
        elif alloc.kind == "ExternalOutput":
            out_names.append(name)
            out_avals.append(jax.core.ShapedArray(
                tuple(alloc.tensor_shape), mybir.dt.np(alloc.dtype)))
    n_params = len(in_names)
    all_names = in_names + out_names + ([pname] if pname else [])

    def _body(*args):
        operands = list(args)
        if pname is not None:
            operands.append(bass2jax.partition_id_tensor())
        outs = bass2jax._bass_exec_p.bind(
            *operands,
            out_avals=tuple(out_avals),
            in_names=tuple(all_names),
            out_names=tuple(out_names),
            lowering_input_output_aliases=(),
            sim_require_finite=False,
            sim_require_nnan=False,
            nc=nc,
        )
        return tuple(outs)

    devices = jax.devices()[:NCORES]
    mesh = Mesh(np.asarray(devices), ("core",))
    spec_in = (PartitionSpec("core"),) * (n_params + len(out_names))
    spec_out = (PartitionSpec("core"),) * len(out_names)
    donate = tuple(range(n_params, n_params + len(out_names)))
    fn = jax.jit(
        shard_map(_body, mesh=mesh, in_specs=spec_in, out_specs=spec_out,
                  check_rep=False),
        donate_argnums=donate, keep_unused=True)

    res = {
        "jax": jax, "mesh": mesh, "NamedSharding": NamedSharding,
        "PartitionSpec": PartitionSpec, "fn": fn, "nc": nc,
        "in_names": in_names, "out_names": out_names, "out_avals": out_avals,
        "devices": devices,
    }
    _EXEC_CACHE[variant] = res
    return res


def _place_inputs(ex, in_maps):
    """Transfer per-core input shards to the 8 devices, return global arrays."""
    jax = ex["jax"]
    NamedSharding, PartitionSpec = ex["NamedSharding"], ex["PartitionSpec"]
    sharding = NamedSharding(ex["mesh"], PartitionSpec("core"))
    placed = []
    for name in ex["in_names"]:
        shards = [np.asarray(in_maps[c][name]) for c in range(NCORES)]
        per_dev = [jax.device_put(s, d) for s, d in zip(shards, ex["devices"])]
        gshape = (NCORES * shards[0].shape[0],) + shards[0].shape[1:]
        placed.append(jax.make_array_from_single_device_arrays(
            gshape, sharding, per_dev))
    jax.block_until_ready(placed)
    return placed, sharding


def _zero_outs(ex, sharding):
    import jax.numpy as jnp
    outs = []
    for av in ex["out_avals"]:
        gshape = (NCORES * av.shape[0],) + av.shape[1:]
        outs.append(jnp.zeros(gshape, av.dtype, device=sharding))
    ex["jax"].block_until_ready(outs)
    return outs


def run_hw(inputs, time_iters=0, variant=None):
    """Run on the 8 NeuronCores. Returns (full_output, wall_times_s)."""
    ex = _get_exec(variant)
    jax = ex["jax"]
    in_maps = in_maps_for(inputs)
    placed, sharding = _place_inputs(ex, in_maps)

    zouts = _zero_outs(ex, sharding)
    res = ex["fn"](*placed, *zouts)
    jax.block_until_ready(res)
    out_global = np.asarray(res[0])          # [8*512, OPAD] bf16

    times = []
    for _ in range(time_iters):
        zouts = _zero_outs(ex, sharding)
        t0 = time.perf_counter()
        r = ex["fn"](*placed, *zouts)
        jax.block_until_ready(r)
        times.append(time.perf_counter() - t0)

    full = out_global[:, :O].astype(np.float32).reshape(T, 1, O)
    return full, times


def kernel(**inputs):
    out, _ = run_hw(inputs, time_iters=0)
    return out


# ---------------------------------------------------------------- dev helpers

def sim_check(inputs, core=0, variant=None):
    """Run core `core`'s program in CoreSim, return its [512, OPAD] output."""
    from concourse.bass_interp import CoreSim
    nc = build_program_real(variant)
    sim = CoreSim(nc, trace=False, require_finite=False, require_nnan=False)
    in_maps = in_maps_for(inputs)
    for name, arr in in_maps[core].items():
        try:
            sim.tensor(name)[:] = arr
        except KeyError:
            pass
    sim.simulate(check_with_hw=False)
    return np.array(sim.tensor("out"))


def timeline(variant=None):
    from concourse.timeline_sim import TimelineSim
    nc = build_program_real(variant)
    tl = TimelineSim(nc, trace=False)
    tl.simulate()
    return tl


def probe_floor(iters=5):
    """Wall-time floor of the 8-core dispatch path using a trivial NEFF."""
    import jax
    from jax.sharding import Mesh, PartitionSpec, NamedSharding
    try:
        from jax.experimental.shard_map import shard_map
    except ImportError:
        from jax import shard_map
    from concourse import bass2jax
    bass2jax.install_neuronx_cc_hook()

    nc = bacc.Bacc("TRN2", target_bir_lowering=False, debug=False,
                   enable_asserts=False)
    pin = nc.dram_tensor("pin", [128, 128], f32, kind="ExternalInput")
    pout = nc.dram_tensor("pout", [128, 128], f32, kind="ExternalOutput")
    with tile.TileContext(nc) as tc:
        with tc.tile_pool(name="p", bufs=1) as pool:
            t = pool.tile([128, 128], f32, tag="t")
            nc.sync.dma_start(t[:], pin.ap()[:])
            nc.sync.dma_start(pout.ap()[:], t[:])
    nc.compile()

    pname = nc.partition_id_tensor.name if nc.partition_id_tensor else None
    all_names = ["pin", "pout"] + ([pname] if pname else [])

    def _body(a, z):
        ops = [a, z]
        if pname is not None:
            ops.append(bass2jax.partition_id_tensor())
        return tuple(bass2jax._bass_exec_p.bind(
            *ops, out_avals=(jax.core.ShapedArray((128, 128), np.float32),),
            in_names=tuple(all_names), out_names=("pout",),
            lowering_input_output_aliases=(),
            sim_require_finite=False, sim_require_nnan=False, nc=nc))

    devices = jax.devices()[:NCORES]
    mesh = Mesh(np.asarray(devices), ("core",))
    sharding = NamedSharding(mesh, PartitionSpec("core"))
    fn = jax.jit(shard_map(_body, mesh=mesh,
                           in_specs=(PartitionSpec("core"),) * 2,
                           out_specs=(PartitionSpec("core"),),
                           check_rep=False), keep_unused=True)
    import jax.numpy as jnp
    a = jax.device_put(np.zeros((NCORES * 128, 128), np.float32), sharding)
    z = jnp.zeros((NCORES * 128, 128), np.float32, device=sharding)
    jax.block_until_ready([a, z])
    r = fn(a, z); jax.block_until_ready(r)   # warm

    def timed(reps):
        best = float("inf")
        for _ in range(iters):
            t0 = time.perf_counter()
            r = None
            for _ in range(reps):
                r = fn(a, z)
            jax.block_until_ready(r)
            best = min(best, time.perf_counter() - t0)
        return best

    w1 = timed(1)
    wk = timed(50)
    return (wk - w1) / 49.0, wk, w1


def run_hw_async(inputs, k=50, iters=3, variant=None):
    """Per-exec time via async pipelining: submit k executions without
    intermediate blocking; marginal cost per call ~= device exec time if the
    runtime queues them. Returns (per_exec_s, wall_k, wall_1)."""
    import jax
    from jax.sharding import PartitionSpec
    try:
        from jax.experimental.shard_map import shard_map
    except ImportError:
        from jax import shard_map
    from concourse import bass2jax
    ex = _get_exec(variant)
    nc = ex["nc"]
    pname = nc.partition_id_tensor.name if nc.partition_id_tensor else None
    in_names, out_names, out_avals = ex["in_names"], ex["out_names"], ex["out_avals"]
    all_names = in_names + out_names + ([pname] if pname else [])

    def _body(*args):
        ops = list(args)
        if pname is not None:
            ops.append(bass2jax.partition_id_tensor())
        return tuple(bass2jax._bass_exec_p.bind(
            *ops, out_avals=tuple(out_avals), in_names=tuple(all_names),
            out_names=tuple(out_names), lowering_input_output_aliases=(),
            sim_require_finite=False, sim_require_nnan=False, nc=nc))

    nin = len(in_names) + len(out_names)
    fn = jax.jit(shard_map(_body, mesh=ex["mesh"],
                           in_specs=(PartitionSpec("core"),) * nin,
                           out_specs=(PartitionSpec("core"),) * len(out_names),
                           check_rep=False), keep_unused=True)  # no donation

    in_maps = in_maps_for(inputs)
    placed, sharding = _place_inputs(ex, in_maps)
    zouts = _zero_outs(ex, sharding)
    r = fn(*placed, *zouts); jax.block_until_ready(r)   # warm

    def timed(reps):
        best = float("inf")
        for _ in range(iters):
            t0 = time.perf_counter()
            r = None
            for _ in range(reps):
                r = fn(*placed, *zouts)
            jax.block_until_ready(r)
            best = min(best, time.perf_counter() - t0)
        return best

    w1 = timed(1)
    wk = timed(k)
    return (wk - w1) / (k - 1), wk, w1


# revision 18
# speedup vs baseline: 1.7338x; 1.1266x over previous
"""Trainium2 Bass kernel for nn_Network_21998822490747 (embedding -> tiny LSTM -> vocab projection).

Strategy (8 NeuronCores, full inputs in / full output out):
  * Time-shard the T=4096 sequence: core c owns rows [c*512, (c+1)*512).
  * The contractive LSTM recurrence runs as S=128 parallel streams (time-chunks
    of L=4 steps) that each start W=16 steps early from zero state; after the
    warmup the state matches the exact scan to fp32 noise. Streams are
    vectorized along SBUF partitions, so one scan step is ~7 engine
    instructions covering all 128 streams.
  * All gate activations use a single tanh per step (sigmoid(x)=0.5(tanh(x/2)+1)
    with scales folded into weights host-side; state kept as C=2c, h2=2h).
  * Memory-bound phase = the [512,10] x [10, 50257] logits matmul per core:
      - W_out/bias split into fp8e4 hi+lo components (host), hs split into
        fp8 hi+lo on device -> fp8 DoubleRow matmuls (2 cols/cycle, K=16x2)
        reproduce bf16-accuracy logits at twice the PE rate.
      - PSUM f32 results are drained to bf16 staging tiles by all three data
        engines (DVE/ACT/Pool, weighted by their throughput), then DMA'd to a
        bf16 output tensor; the host upcasts to f32 (rel-err ~4e-3 << 2e-2).
  * Embedding table stored bf16 on device (halves gather traffic); an appended
    row V (least-squares solution of w_ih @ v = -(b_ih+b_hh)) makes pre-start
    warmup steps exact no-ops so stream 0 starts from the true zero state.
"""

import os
import sys
import time

for _p in ("/opt/trn_rl_repo", "/root/.axon_site/_ro/trn_rl_repo"):
    if os.path.isdir(_p) and _p not in sys.path:
        sys.path.insert(0, _p)

import numpy as np
import ml_dtypes

import concourse.bass as bass
import concourse.bacc as bacc
import concourse.mybir as mybir
import concourse.tile as tile
from concourse.bass import ts
from concourse.masks import make_identity

# Problem shapes
T, V, E, H, O = 4096, 128000, 256, 10, 50257
NCORES = 8
ROWS = T // NCORES        # 512 output rows per core

# Scan decomposition
S = 128                   # parallel streams per core (on SBUF partitions)
L = ROWS // S             # 4 real steps per stream
W = 16                    # warmup steps per stream
STEPS = L + W             # 20
NR = S * STEPS            # 2560 gathered rows per core
CB = NR // 128            # 20 gather column-blocks

# Logits tiling
OPAD = 51200              # O padded to 100 x 512
OC = OPAD // 512          # 512-col chunks per 128-row block
PSB = 2048                # psum drain tile cols (4 chunks / 4 banks)
NPT = OPAD // PSB         # 25 psum tiles per block
STG = 10240               # staging tile cols (5 psum tiles)
NST = OPAD // STG         # 5 staging batches per block

f32 = mybir.dt.float32
bf16 = mybir.dt.bfloat16
fp8 = mybir.dt.float8e4
i32 = mybir.dt.int32
AF = mybir.ActivationFunctionType
AL = mybir.AluOpType
PM = mybir.MatmulPerfMode

NP_BF16 = ml_dtypes.bfloat16
NP_FP8 = ml_dtypes.float8_e4m3

GATE_PERM = np.r_[0:10, 10:20, 30:40, 20:30]   # [i, f, o, g] row order

# drain-engine split: target shares solving for equal per-engine busy time
# given per-tile costs (DVE 2258ns, ACT ~1890ns) and each engine's fixed
# (non-drain) work. GPSIMD/Pool cannot read PSUM on TRN2.
_DRAIN_W = {"v": 0.42, "a": 0.58}


def _drain_pattern(n):
    tot = sum(_DRAIN_W.values())
    acc = {k: 0.0 for k in _DRAIN_W}
    out = []
    for _ in range(n):
        for k in _DRAIN_W:
            acc[k] += _DRAIN_W[k] / tot
        pick = max(acc, key=lambda k: acc[k])
        acc[pick] -= 1.0
        out.append(pick)
    return out


def _tile_kernel(tc, nc, emb, idx, wihT, whhT, b40, wout, out):
    with (
        tc.tile_pool(name="const", bufs=1) as cpool,
        tc.tile_pool(name="work", bufs=1) as wpool,
    ):
        wih_sb = cpool.tile([128, 80], bf16, tag="wih")
        whh_sb = cpool.tile([H, 40], bf16, tag="whh")
        b40_sb = cpool.tile([40, 1], f32, tag="b40")
        ident = cpool.tile([128, 128], f32, tag="ident")
        identb = cpool.tile([128, 128], bf16, tag="identb")
        wout_sb = cpool.tile([16, 2 * OPAD], fp8, tag="wout")
        idx_sb = cpool.tile([128, CB], i32, tag="idx")

        nc.sync.dma_start(wih_sb[:, 0:40], wihT[0:128, :])
        nc.sync.dma_start(wih_sb[:, 40:80], wihT[128:256, :])
        nc.sync.dma_start(whh_sb[:], whhT[:])
        nc.sync.dma_start(b40_sb[:], b40[:])
        nc.sync.dma_start(wout_sb[:], wout[:])
        nc.sync.dma_start(idx_sb[:], idx[:])
        make_identity(nc, ident[:])
        make_identity(nc, identb[:])

        # hs history: h2 (=2h) per step, bf16, streams-major inside each step
        hs = wpool.tile([H, (STEPS + 1) * S], bf16, tag="hs")

        # ---- gather + transpose + xg (gather tiles freed before logits)
        # xg128: streams-on-partitions, step t at cols [t*40, (t+1)*40)
        xg128 = wpool.tile([S, STEPS * 40], f32, tag="xg128")
        with tc.tile_pool(name="gath", bufs=1) as gpool:
            emb_raw = gpool.tile([128, CB * E], bf16, tag="raw")
            for c in range(CB):
                nc.gpsimd.indirect_dma_start(
                    out=emb_raw[:, c * E:(c + 1) * E],
                    out_offset=None,
                    in_=emb[:, :],
                    in_offset=bass.IndirectOffsetOnAxis(
                        ap=idx_sb[:, c:c + 1], axis=0),
                )

            # transpose to emb^T layout [E, NR] (two 128-row halves), bf16
            # gather row j = c*128 + p, so block c is exactly scan step c's
            # streams: the scan can start as soon as early blocks land.
            embT0 = gpool.tile([128, NR], bf16, tag="embT0")
            embT1 = gpool.tile([128, NR], bf16, tag="embT1")
            embTv = [embT0, embT1]
            with tc.tile_pool(name="pst", bufs=2, space="PSUM") as pst:
                for c0 in range(0, CB, 2):
                    for e2 in range(2):
                        ps = pst.tile([128, 256], bf16, tag="tp")
                        for k in range(2):
                            c = c0 + k
                            base = c * E + e2 * 128
                            nc.tensor.transpose(
                                ps[:, ts(k, 128)],
                                emb_raw[:, base:base + 128], identb[:])
                        # all-bf16 contiguous copy: DVE 2x mode applies
                        nc.vector.tensor_copy(
                            embTv[e2][:, c0 * 128:(c0 + 2) * 128], ps[:])

            # xg40 = emb @ w_ih^T + bias (pre-scaled on host) -> [40, NR] f32
            xg40 = gpool.tile([40, NR], f32, tag="xg40")
            with tc.tile_pool(name="psx", bufs=2, space="PSUM") as psx:
                for n in range(NR // 512):
                    ps = psx.tile([40, 512], f32, tag="xg")
                    nc.tensor.matmul(ps[:], lhsT=wih_sb[:, 0:40],
                                     rhs=embT0[:, ts(n, 512)],
                                     start=True, stop=False)
                    nc.tensor.matmul(ps[:], lhsT=wih_sb[:, 40:80],
                                     rhs=embT1[:, ts(n, 512)],
                                     start=False, stop=True)
                    nc.scalar.activation(xg40[:, ts(n, 512)], ps[:], AF.Identity,
                                         bias=b40_sb[:, 0:1], scale=1.0)

            # transpose xg40 step-blocks [40, S] -> xg128 blocks [S, 40]
            with tc.tile_pool(name="psx2", bufs=2, space="PSUM") as psx2:
                for t0 in range(0, STEPS, 4):
                    ps = psx2.tile([S, 160], f32, tag="xt")
                    for k in range(4):
                        nc.tensor.transpose(ps[:, ts(k, 40)],
                                            xg40[:, ts(t0 + k, S)],
                                            ident[0:40, 0:40])
                    eng = nc.vector.tensor_copy if (t0 // 4) % 2 else nc.scalar.copy
                    eng(xg128[:, t0 * 40:(t0 + 4) * 40], ps[:])

        # ---- vectorized scan: STEPS x S streams (streams on partitions)
        # th free-col layout: 0:40 tanh(gates i,f,o,g) | 40:50 C(=2c) | 50:60 tanh(c)
        th = wpool.tile([S, 60], f32, tag="th")
        gt = wpool.tile([S, 40], f32, tag="gt")
        uv = wpool.tile([S, 20], f32, tag="uv")
        h2 = wpool.tile([S, H], f32, tag="h2")
        nc.gpsimd.memset(th[:, 40:50], 0.0)      # C = 2c state
        nc.gpsimd.memset(h2[:, :], 0.0)
        with (
            tc.tile_pool(name="psm", bufs=2, space="PSUM") as psm,
            tc.tile_pool(name="pst2", bufs=2, space="PSUM") as pst2,
        ):
            for t in range(STEPS + 1):
                # hT(t) = h2(t-1)^T  -> the hs history used by logits (bf16)
                pst_ = pst2.tile([H, S], f32, tag="ht")
                nc.tensor.transpose(pst_[:], h2[:, :], ident[0:S, 0:S])
                nc.vector.tensor_copy(hs[:, ts(t, S)], pst_[:])
                if t == STEPS:
                    break
                ps = psm.tile([S, 40], f32, tag="mv")
                nc.tensor.matmul(ps[:], lhsT=hs[:, ts(t, S)], rhs=whh_sb[:],
                                 start=True, stop=True)
                nc.vector.scalar_tensor_tensor(gt[:, :], ps[:], 1.0,
                                               xg128[:, ts(t, 40)], AL.mult, AL.add)
                nc.scalar.activation(th[:, 0:40], gt[:, :], AF.Tanh)
                # u = (th_i+1)*th_g ; v = (th_f+1)*C   (one fused op)
                nc.vector.scalar_tensor_tensor(uv[:, :], th[:, 0:20], 1.0,
                                               th[:, 30:50], AL.add, AL.mult)
                nc.vector.scalar_tensor_tensor(th[:, 40:50], uv[:, 10:20], 0.5,
                                               uv[:, 0:10], AL.mult, AL.add)
                nc.scalar.activation(th[:, 50:60], th[:, 40:50], AF.Tanh, scale=0.5)
                nc.vector.scalar_tensor_tensor(h2[:, :], th[:, 20:30], 1.0,
                                               th[:, 50:60], AL.add, AL.mult)

        # ---- fp8 hi/lo split of hs output rows (h2 = h8 + d8)
        # output row r = s*L + u  <-  hs step W+1+u, stream s
        hs_r = hs[:].rearrange("p (t s) -> p s t", s=S)[:, :, W + 1:W + 1 + L]
        h8 = wpool.tile([H, ROWS], fp8, tag="h8")
        h8b = wpool.tile([H, ROWS], bf16, tag="h8b")
        d8 = wpool.tile([H, ROWS], fp8, tag="d8")
        statq = wpool.tile([16, 4 * 256], fp8, tag="statq")
        nc.vector.tensor_copy(h8[:], hs_r)
        nc.vector.tensor_copy(h8b[:], h8[:])
        nc.vector.tensor_tensor(d8[:], hs_r, h8b[:], AL.subtract)
        # statq layout: [16, (blk 4, i 2, m 128)]; memset 1.0 covers bias rows
        # (SBUF-only assembly runs on Pool to keep DVE/ACT free for drains)
        nc.gpsimd.memset(statq[:, :], 1.0)
        st4 = statq[:].rearrange("p (b i m) -> p b i m", i=2, m=128)
        h8_4 = h8[:].rearrange("p (b m) -> p b m", m=128)
        d8_4 = d8[:].rearrange("p (b m) -> p b m", m=128)
        nc.gpsimd.tensor_copy(st4[0:10, :, 0, :], h8_4)
        nc.gpsimd.tensor_copy(st4[0:10, :, 1, :], h8_4)
        # engines can't start at partition 11; DMA has no such restriction
        nc.sync.dma_start(st4[11:16, :, 0, :], d8_4[0:5])
        nc.sync.dma_start(st4[11:16, :, 1, :], d8_4[5:10])

        # ---- logits: fp8 DoubleRow matmuls, DVE/ACT drain, bf16 DMA out
        # wout packed per 512-chunk: [16, (oc, i, 512)] so AP strides stay
        # within the 16-bit ISA field (a global i-stride of OPAD=51200 is not
        # encodable)
        drain_eng = {"v": nc.vector.tensor_copy,
                     "a": nc.scalar.copy}
        pattern = _drain_pattern(4 * NPT)
        pi = 0
        with (
            tc.tile_pool(name="psl", bufs=2, space="PSUM") as psl,
            tc.tile_pool(name="stage", bufs=3) as stpool,
        ):
            for blk in range(ROWS // 128):
                lhsT = statq[:, blk * 256:(blk + 1) * 256].rearrange(
                    "p (i m) -> p i m", i=2)
                stage = None
                for pt in range(NPT):
                    ps = psl.tile([128, PSB], f32, tag="lg")
                    for k in range(4):
                        oc = pt * 4 + k
                        rhs = wout_sb[:, oc * 1024:(oc + 1) * 1024].rearrange(
                            "p (i n) -> p i n", i=2)
                        nc.tensor.matmul(
                            ps[:, ts(k, 512)], lhsT=lhsT,
                            rhs=rhs,
                            start=True, stop=True, perf_mode=PM.DoubleRow)
                    if pt % 5 == 0:
                        stage = stpool.tile([128, STG], bf16, tag="stg")
                    drain_eng[pattern[pi]](stage[:, ts(pt % 5, PSB)], ps[:])
                    pi += 1
                    if pt % 5 == 4:
                        col = (pt // 5) * STG
                        nc.sync.dma_start(
                            out[ts(blk, 128), col:col + STG], stage[:])


def build_program_real(variant=None):
    nc = bacc.Bacc("TRN2", target_bir_lowering=False, debug=False,
                   enable_asserts=False)
    emb_ap = nc.dram_tensor("emb", [V + 1, E], bf16, kind="ExternalInput").ap()
    idx_ap = nc.dram_tensor("idx", [128, CB], i32, kind="ExternalInput").ap()
    wih_d = nc.dram_tensor("wihT", [E, 40], bf16, kind="ExternalInput")
    whh_d = nc.dram_tensor("whhT05", [H, 40], bf16, kind="ExternalInput")
    b40_d = nc.dram_tensor("bias40", [40, 1], f32, kind="ExternalInput")
    wout_d = nc.dram_tensor("wout", [16, 2 * OPAD], fp8, kind="ExternalInput")
    out_d = nc.dram_tensor("out", [ROWS, OPAD], bf16, kind="ExternalOutput")

    with tile.TileContext(nc) as tc:
        _tile_kernel(tc, nc, emb_ap, idx_ap, wih_d.ap(), whh_d.ap(),
                     b40_d.ap(), wout_d.ap(), out_d.ap())
    nc.compile()
    return nc


def prep_host(inputs):
    """Shared (core-independent) prepped arrays + per-core index tables."""
    x = np.asarray(inputs["x"]).astype(np.int64)
    embedding = np.asarray(inputs["embedding"], dtype=np.float32)
    w_ih = np.asarray(inputs["w_ih"], dtype=np.float32)
    w_hh = np.asarray(inputs["w_hh"], dtype=np.float32)
    b_ih = np.asarray(inputs["b_ih"], dtype=np.float32)
    b_hh = np.asarray(inputs["b_hh"], dtype=np.float32)
    W_out = np.asarray(inputs["W_out"], dtype=np.float32)
    b_out = np.asarray(inputs["b_out"], dtype=np.float32)

    p = GATE_PERM
    # gate scale: sigmoid(x) = 0.5*(tanh(x/2)+1) -> scale i,f,o preacts by 0.5,
    # folded into w_ih / bias; w_hh additionally gets the h2=2h factor (x0.5).
    gsc = np.concatenate([np.full(30, 0.5), np.ones(10)]).astype(np.float32)
    w_ih_p = w_ih[p] * gsc[:, None]
    bias40 = ((b_ih + b_hh)[p] * gsc).astype(np.float32)
    whh05 = (w_hh[p].T * (0.5 * gsc)[None, :]).astype(NP_BF16)     # [10, 40]
    wihT = np.ascontiguousarray(w_ih_p.T).astype(NP_BF16)          # [256, 40]

    # Padding row V: w_ih @ v = -(b_ih + b_hh)  => xg row == 0 for padded steps
    v, *_ = np.linalg.lstsq(w_ih.astype(np.float64),
                            -(b_ih + b_hh).astype(np.float64), rcond=None)
    emb_aug = np.concatenate(
        [embedding, v[None, :].astype(np.float32)], axis=0).astype(NP_BF16)

    # fp8 hi/lo split of 0.5*W_out (h2=2h folds the 2x) and b_out
    Wt = np.zeros((OPAD, H), np.float32)
    Wt[:O] = 0.5 * W_out
    bo = np.zeros(OPAD, np.float32)
    bo[:O] = b_out
    W_hi = Wt.astype(NP_FP8)
    W_lo = (Wt - W_hi.astype(np.float32)).astype(NP_FP8)
    b_hi = bo.astype(NP_FP8)
    b_lo = (bo - b_hi.astype(np.float32)).astype(NP_FP8)
    # woutp[p, i, n]: i=0 {rows0-9: W_hi, 10: b_hi, 11-15: W_hi[:, :5] (d8 pair)}
    #                i=1 {rows0-9: W_lo, 10: b_lo, 11-15: W_hi[:, 5:]}
    woutp = np.zeros((16, 2, OPAD), NP_FP8)
    woutp[0:10, 0, :] = W_hi.T
    woutp[10, 0, :] = b_hi
    woutp[11:16, 0, :] = W_hi.T[0:5]
    woutp[0:10, 1, :] = W_lo.T
    woutp[10, 1, :] = b_lo
    woutp[11:16, 1, :] = W_hi.T[5:10]
    # interleave hi/lo per 512-col chunk: [16, (oc, i, 512)]
    woutp = np.ascontiguousarray(
        woutp.reshape(16, 2, OPAD // 512, 512).transpose(0, 2, 1, 3)
    ).reshape(16, 2 * OPAD)

    idx_cores = []
    for c in range(NCORES):
        j = np.arange(NR)
        t = j // S
        s = j % S
        g_r = c * ROWS + s * L - W + t
        val = np.where(g_r < 0, V, x[np.clip(g_r, 0, T - 1)])
        # tile position (p, cb) holds gather row j = cb*128 + p
        idx_cores.append(np.ascontiguousarray(
            val.reshape(CB, 128).T).astype(np.int32))

    shared = {
        "emb": emb_aug,
        "wihT": wihT,
        "whhT05": whh05,
        "bias40": np.asarray(bias40).reshape(40, 1),
        "wout": woutp,
    }
    return shared, idx_cores


def in_maps_for(inputs):
    shared, idx_cores = prep_host(inputs)
    return [{**shared, "idx": idx_cores[c]} for c in range(NCORES)]


_EXEC_CACHE = {}


def _get_exec(variant=None):
    """Build (once) the compiled 8-core PJRT executable and metadata."""
    if variant in _EXEC_CACHE:
        return _EXEC_CACHE[variant]

    import jax
    from jax.sharding import Mesh, PartitionSpec, NamedSharding
    try:
        from jax.experimental.shard_map import shard_map
    except ImportError:
        from jax import shard_map
    from concourse import bass2jax

    bass2jax.install_neuronx_cc_hook()
    nc = build_program_real(variant)

    pname = nc.partition_id_tensor.name if nc.partition_id_tensor else None
    in_names, out_names, out_avals = [], [], []
    for alloc in nc.m.functions[0].allocations:
        if not isinstance(alloc, mybir.MemoryLocationSet):
            continue
        name = alloc.memorylocations[0].name
        if alloc.kind == "ExternalInput":
            if name != pname:
                in_names.append(name)
        elif alloc.kind == "ExternalOutput":
            out_names.append(name)
            out_avals.append(jax.core.ShapedArray(
                tuple(alloc.tensor_shape), mybir.dt.np(alloc.dtype)))
    n_params = len(in_names)
    all_names = in_names + out_names + ([pname] if pname else [])

    def _body(*args):
        operands = list(args)
        if pname is not None:
            operands.append(bass2jax.partition_id_tensor())
        outs = bass2jax._bass_exec_p.bind(
            *operands,
            out_avals=tuple(out_avals),
            in_names=tuple(all_names),
            out_names=tuple(out_names),
            lowering_input_output_aliases=(),
            sim_require_finite=False,
            sim_require_nnan=False,
            nc=nc,
        )
        return tuple(outs)

    devices = jax.devices()[:NCORES]
    mesh = Mesh(np.asarray(devices), ("core",))
    spec_in = (PartitionSpec("core"),) * (n_params + len(out_names))
    spec_out = (PartitionSpec("core"),) * len(out_names)
    donate = tuple(range(n_params, n_params + len(out_names)))
    fn = jax.jit(
        shard_map(_body, mesh=mesh, in_specs=spec_in, out_specs=spec_out,
                  check_rep=False),
        donate_argnums=donate, keep_unused=True)

    res = {
        "jax": jax, "mesh": mesh, "NamedSharding": NamedSharding,
        "PartitionSpec": PartitionSpec, "fn": fn, "nc": nc,
        "in_names": in_names, "out_names": out_names, "out_avals": out_avals,
        "devices": devices,
    }
    _EXEC_CACHE[variant] = res
    return res


def _place_inputs(ex, in_maps):
    """Transfer per-core input shards to the 8 devices, return global arrays."""
    jax = ex["jax"]
    NamedSharding, PartitionSpec = ex["NamedSharding"], ex["PartitionSpec"]
    sharding = NamedSharding(ex["mesh"], PartitionSpec("core"))
    placed = []
    for name in ex["in_names"]:
        shards = [np.asarray(in_maps[c][name]) for c in range(NCORES)]
        per_dev = [jax.device_put(s, d) for s, d in zip(shards, ex["devices"])]
        gshape = (NCORES * shards[0].shape[0],) + shards[0].shape[1:]
        placed.append(jax.make_array_from_single_device_arrays(
            gshape, sharding, per_dev))
    jax.block_until_ready(placed)
    return placed, sharding


def _zero_outs(ex, sharding):
    import jax.numpy as jnp
    outs = []
    for av in ex["out_avals"]:
        gshape = (NCORES * av.shape[0],) + av.shape[1:]
        outs.append(jnp.zeros(gshape, av.dtype, device=sharding))
    ex["jax"].block_until_ready(outs)
    return outs


def run_hw(inputs, time_iters=0, variant=None):
    """Run on the 8 NeuronCores. Returns (full_output, wall_times_s)."""
    ex = _get_exec(variant)
    jax = ex["jax"]
    in_maps = in_maps_for(inputs)
    placed, sharding = _place_inputs(ex, in_maps)

    zouts = _zero_outs(ex, sharding)
    res = ex["fn"](*placed, *zouts)
    jax.block_until_ready(res)
    out_global = np.asarray(res[0])          # [8*512, OPAD] bf16

    times = []
    for _ in range(time_iters):
        zouts = _zero_outs(ex, sharding)
        t0 = time.perf_counter()
        r = ex["fn"](*placed, *zouts)
        jax.block_until_ready(r)
        times.append(time.perf_counter() - t0)

    full = out_global[:, :O].astype(np.float32).reshape(T, 1, O)
    return full, times


def kernel(**inputs):
    out, _ = run_hw(inputs, time_iters=0)
    return out


# ---------------------------------------------------------------- dev helpers

def sim_check(inputs, core=0, variant=None):
    """Run core `core`'s program in CoreSim, return its [512, OPAD] output."""
    from concourse.bass_interp import CoreSim
    nc = build_program_real(variant)
    sim = CoreSim(nc, trace=False, require_finite=False, require_nnan=False)
    in_maps = in_maps_for(inputs)
    for name, arr in in_maps[core].items():
        try:
            sim.tensor(name)[:] = arr
        except KeyError:
            pass
    sim.simulate(check_with_hw=False)
    return np.array(sim.tensor("out"))


def timeline(variant=None):
    from concourse.timeline_sim import TimelineSim
    nc = build_program_real(variant)
    tl = TimelineSim(nc, trace=False)
    tl.simulate()
    return tl


def probe_floor(iters=5):
    """Wall-time floor of the 8-core dispatch path using a trivial NEFF."""
    import jax
    from jax.sharding import Mesh, PartitionSpec, NamedSharding
    try:
        from jax.experimental.shard_map import shard_map
    except ImportError:
        from jax import shard_map
    from concourse import bass2jax
    bass2jax.install_neuronx_cc_hook()

    nc = bacc.Bacc("TRN2", target_bir_lowering=False, debug=False,
                   enable_asserts=False)
    pin = nc.dram_tensor("pin", [128, 128], f32, kind="ExternalInput")
    pout = nc.dram_tensor("pout", [128, 128], f32, kind="ExternalOutput")
    with tile.TileContext(nc) as tc:
        with tc.tile_pool(name="p", bufs=1) as pool:
            t = pool.tile([128, 128], f32, tag="t")
            nc.sync.dma_start(t[:], pin.ap()[:])
            nc.sync.dma_start(pout.ap()[:], t[:])
    nc.compile()

    pname = nc.partition_id_tensor.name if nc.partition_id_tensor else None
    all_names = ["pin", "pout"] + ([pname] if pname else [])

    def _body(a, z):
        ops = [a, z]
        if pname is not None:
            ops.append(bass2jax.partition_id_tensor())
        return tuple(bass2jax._bass_exec_p.bind(
            *ops, out_avals=(jax.core.ShapedArray((128, 128), np.float32),),
            in_names=tuple(all_names), out_names=("pout",),
            lowering_input_output_aliases=(),
            sim_require_finite=False, sim_require_nnan=False, nc=nc))

    devices = jax.devices()[:NCORES]
    mesh = Mesh(np.asarray(devices), ("core",))
    sharding = NamedSharding(mesh, PartitionSpec("core"))
    fn = jax.jit(shard_map(_body, mesh=mesh,
                           in_specs=(PartitionSpec("core"),) * 2,
                           out_specs=(PartitionSpec("core"),),
                           check_rep=False), keep_unused=True)
    import jax.numpy as jnp
    a = jax.device_put(np.zeros((NCORES * 128, 128), np.float32), sharding)
    z = jnp.zeros((NCORES * 128, 128), np.float32, device=sharding)
    jax.block_until_ready([a, z])
    r = fn(a, z); jax.block_until_ready(r)   # warm

    def timed(reps):
        best = float("inf")
        for _ in range(iters):
            t0 = time.perf_counter()
            r = None
            for _ in range(reps):
                r = fn(a, z)
            jax.block_until_ready(r)
            best = min(best, time.perf_counter() - t0)
        return best

    w1 = timed(1)
    wk = timed(50)
    return (wk - w1) / 49.0, wk, w1


def run_hw_async(inputs, k=50, iters=3, variant=None):
    """Per-exec time via async pipelining: submit k executions without
    intermediate blocking; marginal cost per call ~= device exec time if the
    runtime queues them. Returns (per_exec_s, wall_k, wall_1)."""
    import jax
    from jax.sharding import PartitionSpec
    try:
        from jax.experimental.shard_map import shard_map
    except ImportError:
        from jax import shard_map
    from concourse import bass2jax
    ex = _get_exec(variant)
    nc = ex["nc"]
    pname = nc.partition_id_tensor.name if nc.partition_id_tensor else None
    in_names, out_names, out_avals = ex["in_names"], ex["out_names"], ex["out_avals"]
    all_names = in_names + out_names + ([pname] if pname else [])

    def _body(*args):
        ops = list(args)
        if pname is not None:
            ops.append(bass2jax.partition_id_tensor())
        return tuple(bass2jax._bass_exec_p.bind(
            *ops, out_avals=tuple(out_avals), in_names=tuple(all_names),
            out_names=tuple(out_names), lowering_input_output_aliases=(),
            sim_require_finite=False, sim_require_nnan=False, nc=nc))

    nin = len(in_names) + len(out_names)
    fn = jax.jit(shard_map(_body, mesh=ex["mesh"],
                           in_specs=(PartitionSpec("core"),) * nin,
                           out_specs=(PartitionSpec("core"),) * len(out_names),
                           check_rep=False), keep_unused=True)  # no donation

    in_maps = in_maps_for(inputs)
    placed, sharding = _place_inputs(ex, in_maps)
    zouts = _zero_outs(ex, sharding)
    r = fn(*placed, *zouts); jax.block_until_ready(r)   # warm

    def timed(reps):
        best = float("inf")
        for _ in range(iters):
            t0 = time.perf_counter()
            r = None
            for _ in range(reps):
                r = fn(*placed, *zouts)
            jax.block_until_ready(r)
            best = min(best, time.perf_counter() - t0)
        return best

    w1 = timed(1)
    wk = timed(k)
    return (wk - w1) / (k - 1), wk, w1
